# revision 1
# baseline (speedup 1.0000x reference)
"""Trainium2 Bass kernel for nn_MLA_28793460752680 (MLA attention block).

Sharding: 8 cores = (batch b in 0..1) x (head-group g in 0..3, 4 heads each).
Each core computes h = x[b] @ w1 redundantly (x4), then only its head-group's
projections + attention + a partial output projection; host sums partials.

Device layouts are feature-major (transposed): hT [1024, T]; per-head q/k
assembled as [128, head, T] with 64 latent rows on partitions 0:64 and 64
rope rows on 64:128; v token-major [T, 4, 65] with a ones column fused so
the PV matmul also produces the softmax denominators; attention PV output is
token-major and PE-transposed before the wo projection. RoPE cos/sin tables,
causal masks, and the pair-swap permutation are precomputed on the host.
"""
import sys
sys.path.insert(0, '/opt/trn_rl_repo')
import numpy as np

B, T, C = 2, 2048, 1024
NH, LAT, DHR = 16, 512, 64
DK = 64
P = 128
SCALE = float((DK + DHR) ** -0.5)
F32R = False   # use float32r (full-rate fp32) for large matmuls
DVE_COPIES = False  # route PSUM->SBUF bias-copies to DVE instead of ACT

_BUILT = {}


# ---------------------------------------------------------------- host tables
def _rope_tables(d):
    freq = np.arange(T, dtype=np.float64)[:, None] + 1.0
    pos = np.arange(d // 2, dtype=np.float64)[:, None]
    pos = np.repeat(pos, 2, axis=-1).reshape(1, -1)
    theta = np.exp(-2.0 * pos / d * np.log(10000.0))
    cos = np.cos(freq * theta)
    sin = np.sin(freq * theta)
    sgn = np.tile(np.array([-1.0, 1.0]), d // 2)[None, :]
    return cos.astype(np.float32), (sin * sgn).astype(np.float32)


def _masks_packed():
    m = np.zeros((P, 4, 512), np.float32)
    for j in range(4):
        tk = j * P + np.arange(P)[:, None]
        f = np.arange(512)[None, :]
        m[:, j, :] = (tk <= f).astype(np.float32)
    return m


def _pairswap():
    s = np.zeros((P, P), np.float32)
    for k in range(P):
        s[k, k ^ 1] = 1.0
    return s


# ---------------------------------------------------------------- device prog
def _build_program():
    import concourse.mybir as mybir
    import concourse.tile as tile
    from concourse import bacc
    from concourse.masks import make_identity

    NCH = T // 512
    f32 = mybir.dt.float32
    wdt = mybir.dt.float32r if F32R else mybir.dt.float32
    AF = mybir.ActivationFunctionType
    nc = bacc.Bacc(None, target_bir_lowering=False, debug=False)

    def rmm(out, lhsT, rhs, **kw):
        nc.tensor.matmul(out, lhsT, rhs, **kw)

    def bias_copy(out, in_, bias_ap):
        # PSUM -> SBUF eviction with per-partition bias add
        if DVE_COPIES:
            n = out.shape[-1]
            p = out.shape[0]
            nc.vector.tensor_add(out, in_,
                                 bias_ap.to_broadcast((p, n)))
        else:
            nc.scalar.activation(out, in_, AF.Identity, bias=bias_ap)

    def din(name, shape, dt=None):
        return nc.declare_dram_parameter(name, list(shape), dt or f32,
                                         isOutput=False)

    xT = din('xT', (8, P, T), wdt)                 # x[b].T grouped [ko,p,t]
    w1 = din('w1', (8, P, C), wdt)                 # [K=1024 grouped, M=1024]
    b1 = din('b1', (P, 8))
    wkr = din('wkr', (8, P, 2 * DHR), wdt)
    bkr = din('bkr', (P, 1))                  # rows 64:128 hold bkr, rest 0
    wqr = din('wqr', (8, P, 256), wdt)
    bqr = din('bqr', (P, 2))
    wkvk = din('wkvk', (4, P, 256), wdt)
    bkvk = din('bkvk', (P, 2))
    wkvv = din('wkvv', (4, P, 256), wdt)
    vinit = din('vinit', (P, 4, 65))
    wq = din('wq', (4, P, 256), wdt)
    bq = din('bq', (P, 2))
    wo = din('wo', (2, P, C), wdt)
    cos_qr = din('cos_qr', (2, P, T), wdt)
    sin_qr = din('sin_qr', (2, P, T), wdt)
    cos_kr = din('cos_kr', (P, T), wdt)            # rows 64:128 hold table, rest 0
    sin_kr = din('sin_kr', (P, T), wdt)
    masks = din('masks', (P, 4, 512))
    sperm = din('sperm', (P, P), wdt)
    partial = nc.declare_dram_parameter('partial', [T // P, P, C], f32,
                                        isOutput=True)

    with tile.TileContext(nc) as tc:
        with (
            tc.tile_pool(name='const', bufs=1) as const,
            tc.tile_pool(name='qk', bufs=1) as qkpool,
            tc.tile_pool(name='vpool', bufs=1) as vpool,
        ):
            # small constants
            S = const.tile([P, P], wdt)
            nc.sync.dma_start(S[:], sperm[:])
            ident = const.tile([P, P], f32)
            make_identity(nc, ident)
            b1_sb = const.tile([P, 8], f32)
            nc.sync.dma_start(b1_sb[:], b1[:])
            bkr_sb = const.tile([P, 1], f32)
            nc.sync.dma_start(bkr_sb[:], bkr[:])
            bqr_sb = const.tile([P, 2], f32)
            nc.sync.dma_start(bqr_sb[:], bqr[:])
            bkvk_sb = const.tile([P, 2], f32)
            nc.sync.dma_start(bkvk_sb[:], bkvk[:])
            bq_sb = const.tile([P, 2], f32)
            nc.sync.dma_start(bq_sb[:], bq[:])

            with (
                tc.tile_pool(name='hTp', bufs=1) as hTp,
                tc.tile_pool(name='psA', bufs=3, space='PSUM') as psA,
            ):
                hT = hTp.tile([P, 8, T], wdt)

                # ---------------- phase 1: hT = w1.T @ xT + b1 -------------
                with (
                    tc.tile_pool(name='w1p', bufs=1) as w1p,
                    tc.tile_pool(name='xs', bufs=1) as xs,
                ):
                    w1_sb = w1p.tile([P, 8, C], wdt)
                    for ko in range(8):
                        nc.sync.dma_start(w1_sb[:, ko], w1[ko])
                    for nch in range(NCH):
                        sl = slice(nch * 512, (nch + 1) * 512)
                        xc = xs.tile([P, 8, 512], wdt, tag='xc')
                        for ko in range(8):
                            nc.sync.dma_start(xc[:, ko], xT[ko, :, sl])
                        for m in range(8):
                            ps = psA.tile([P, 512], f32, tag='proj')
                            for ko in range(8):
                                rmm(ps[:],
                                                 w1_sb[:, ko, m * P:(m + 1) * P],
                                                 xc[:, ko],
                                                 start=(ko == 0), stop=(ko == 7))
                            bias_copy(hT[:, m, sl], ps[:], b1_sb[:, m:m + 1])

                # ---------- phase 2a: rope projections (kRt, qRt) ----------
                q_sb = qkpool.tile([P, 4, T], wdt)  # [0:64]=qT(h) [64:128]=qRt
                k_sb = qkpool.tile([P, 4, T], wdt)  # [0:64]=kT(h) [64:128]=kRt
                with (
                    tc.tile_pool(name='wrope', bufs=1) as wrp,
                    tc.tile_pool(name='tabs', bufs=1) as tabs,
                    tc.tile_pool(name='stage', bufs=2) as stage,
                ):
                    wkr_sb = wrp.tile([P, 8, 2 * DHR], wdt)
                    for ko in range(8):
                        nc.sync.dma_start(wkr_sb[:, ko], wkr[ko])
                    wqr_sb = wrp.tile([P, 8, 256], wdt)
                    for ko in range(8):
                        nc.sync.dma_start(wqr_sb[:, ko], wqr[ko])

                    for nch in range(NCH):
                        sl = slice(nch * 512, (nch + 1) * 512)
                        ckr = tabs.tile([P, 512], wdt, tag='ckr')
                        skr = tabs.tile([P, 512], wdt, tag='skr')
                        nc.sync.dma_start(ckr[64:128], cos_kr[64:128, sl])
                        nc.sync.dma_start(skr[64:128], sin_kr[64:128, sl])
                        cqr = tabs.tile([P, 2, 512], wdt, tag='cqr')
                        sqr = tabs.tile([P, 2, 512], wdt, tag='sqr')
                        for ko in range(2):
                            nc.sync.dma_start(cqr[:, ko], cos_qr[ko, :, sl])
                            nc.sync.dma_start(sqr[:, ko], sin_qr[ko, :, sl])

                        # kRt lives on partitions 64:128 throughout
                        ps = psA.tile([P, 512], f32, tag='proj')
                        for ko in range(8):
                            rmm(ps[:], wkr_sb[:, ko], hT[:, ko, sl],
                                start=(ko == 0), stop=(ko == 7))
                        raw = stage.tile([P, 512], wdt, tag='raw')
                        bias_copy(raw[64:128], ps[64:128], bkr_sb[64:128])
                        sw = psA.tile([P, 512], f32, tag='swap')
                        nc.tensor.matmul(sw[64:128],
                                         S[64:128, 64:128].bitcast(f32),
                                         raw[64:128].bitcast(f32),
                                         start=True, stop=True)
                        t1 = stage.tile([P, 512], wdt, tag='t1')
                        nc.vector.tensor_mul(t1[64:128], raw[64:128], ckr[64:128])
                        nc.vector.tensor_mul(raw[64:128], sw[64:128], skr[64:128])
                        for h in range(4):
                            nc.vector.tensor_add(k_sb[64:128, h, sl],
                                                 t1[64:128], raw[64:128])

                        # qRt: m covers heads 2m (rows 0:64), 2m+1 (64:128)
                        for m in range(2):
                            ps = psA.tile([P, 512], f32, tag='proj')
                            for ko in range(8):
                                rmm(ps[:],
                                                 wqr_sb[:, ko, m * P:(m + 1) * P],
                                                 hT[:, ko, sl],
                                                 start=(ko == 0), stop=(ko == 7))
                            raw = stage.tile([P, 512], wdt, tag='raw')
                            bias_copy(raw[:], ps[:], bqr_sb[:, m:m + 1])
                            sw = psA.tile([P, 512], f32, tag='swap')
                            rmm(sw[:], S[:], raw[:],
                                             start=True, stop=True)
                            t1 = stage.tile([P, 512], wdt, tag='t1')
                            nc.vector.tensor_mul(t1[:], raw[:], cqr[:, m])
                            nc.vector.tensor_mul(raw[:], sw[:], sqr[:, m])
                            # odd head 2m+1 (rows 64:128): aligned direct add
                            nc.vector.tensor_add(q_sb[64:128, 2 * m + 1, sl],
                                                 t1[64:128], raw[64:128])
                            # even head 2m: add at 0:64, DMA-shift down
                            t2 = stage.tile([P, 512], wdt, tag='t2')
                            nc.vector.tensor_add(t2[0:64], t1[0:64], raw[0:64])
                            nc.sync.dma_start(q_sb[64:128, 2 * m, sl], t2[0:64])

                # ---------- phase 2b: kT, qT, v ----------
                with (
                    tc.tile_pool(name='w2', bufs=1) as w2p,
                    tc.tile_pool(name='stage2', bufs=3) as stage2,
                ):
                    wkvk_sb = w2p.tile([P, 4, 256], wdt)
                    wkvv_sb = w2p.tile([P, 4, 256], wdt)
                    wq_sb = w2p.tile([P, 4, 256], wdt)
                    for ko in range(4):
                        nc.sync.dma_start(wkvk_sb[:, ko], wkvk[ko])
                        nc.sync.dma_start(wkvv_sb[:, ko], wkvv[ko])
                        nc.sync.dma_start(wq_sb[:, ko], wq[ko])
                    v_sb = vpool.tile([P, T // P, 4, 65], f32)
                    for tt in range(T // P):
                        nc.sync.dma_start(v_sb[:, tt], vinit[:])

                    for nch in range(NCH):
                        sl = slice(nch * 512, (nch + 1) * 512)
                        # kT/qT: 256 rows -> m in {0,1}; cKVT = hT ko 0:4,
                        # cqT = hT ko 4:8
                        for (dst, wsb, bsb, koff) in (
                                (k_sb, wkvk_sb, bkvk_sb, 0),
                                (q_sb, wq_sb, bq_sb, 4)):
                            for m in range(2):
                                ps = psA.tile([P, 512], f32, tag='proj')
                                for ko in range(4):
                                    rmm(
                                        ps[:], wsb[:, ko, m * P:(m + 1) * P],
                                        hT[:, ko + koff, sl],
                                        start=(ko == 0), stop=(ko == 3))
                                # even head 2m: rows 0:64 aligned
                                bias_copy(dst[0:64, 2 * m, sl], ps[0:64],
                                          bsb[0:64, m:m + 1])
                                # odd head 2m+1: rows 64:128, DMA-shift up
                                st = stage2.tile([P, 512], wdt, tag='shift')
                                bias_copy(st[64:128], ps[64:128],
                                          bsb[64:128, m:m + 1])
                                nc.sync.dma_start(dst[0:64, 2 * m + 1, sl],
                                                  st[64:128])
                        # v: token-major, tokens on partitions
                        for mt in range(4):
                            tt = nch * 4 + mt
                            ps = psA.tile([P, 256], f32, tag='swap')
                            for ko in range(4):
                                rmm(
                                    ps[:, 0:256],
                                    hT[:, ko, tt * P:(tt + 1) * P],
                                    wkvv_sb[:, ko],
                                    start=(ko == 0), stop=(ko == 3))
                            nc.vector.tensor_add(
                                v_sb[:, tt, :, 0:64],
                                v_sb[:, tt, :, 0:64],
                                ps[:, 0:256].rearrange('p (h d) -> p h d', d=64))

            # ---------------- phase 3: attention ----------------
            with tc.tile_pool(name='att', bufs=1) as attp:
                att_sb = attp.tile([P, T // P, 256], f32)   # token-major
                with (
                    tc.tile_pool(name='mp', bufs=1) as mp,
                    tc.tile_pool(name='esb', bufs=4) as esb,
                    tc.tile_pool(name='psS', bufs=3, space='PSUM') as psS,
                    tc.tile_pool(name='psO', bufs=1, space='PSUM') as psO,
                    tc.tile_pool(name='ep', bufs=4) as ep,
                ):
                    mask_sb = mp.tile([P, 4, 512], f32)
                    nc.sync.dma_start(mask_sb[:], masks[:])

                    for h in range(4):
                        for sq in range(T // 512):
                            sl = slice(sq * 512, (sq + 1) * 512)
                            ops = [psO.tile([P, 65], f32, tag=f'o{t}', name=f'o{t}')
                                   for t in range(4)]
                            for kt in range(4 * sq + 4):
                                ps = psS.tile([P, 512], f32, tag='score')
                                rmm(ps[:],
                                                 k_sb[:, h, kt * P:(kt + 1) * P],
                                                 q_sb[:, h, sl],
                                                 start=True, stop=True)
                                e = esb.tile([P, 512], f32, tag='e')
                                nc.scalar.activation(e[:], ps[:], AF.Exp,
                                                     scale=SCALE)
                                j = kt - 4 * sq
                                if j >= 0:
                                    nc.vector.tensor_mul(e[:], e[:],
                                                         mask_sb[:, j])
                                for t in range(max(0, j), 4):
                                    rmm(
                                        ops[t][:], e[:, t * P:(t + 1) * P],
                                        v_sb[:, kt, h, :],
                                        start=(kt == 0),
                                        stop=(kt == 4 * sq + t))
                            for t in range(4):
                                tt = sq * 4 + t
                                r = ep.tile([P, 1], f32, tag='recip')
                                nc.vector.reciprocal(r[:], ops[t][:, 64:65])
                                nc.vector.tensor_mul(
                                    att_sb[:, tt, h * 64:(h + 1) * 64],
                                    ops[t][:, 0:64],
                                    r[:, 0:1].to_broadcast((P, 64)))

                # ---------------- phase 4: out = attT @ wo ----------------
                with (
                    tc.tile_pool(name='wop', bufs=1) as wop,
                    tc.tile_pool(name='attT', bufs=1) as attTp,
                    tc.tile_pool(name='outs', bufs=3) as outs,
                    tc.tile_pool(name='psC', bufs=2, space='PSUM') as psC,
                ):
                    wo_sb = wop.tile([P, 2, C], wdt)
                    for ko in range(2):
                        nc.sync.dma_start(wo_sb[:, ko], wo[ko])
                    attT = attTp.tile([P, 2, T], wdt)
                    for tt in range(T // P):
                        for fo in range(2):
                            pst = psC.tile([P, P], f32, tag='tp')
                            nc.tensor.transpose(
                                pst[:], att_sb[:, tt, fo * P:(fo + 1) * P],
                                ident[:])
                            nc.scalar.activation(attT[:, fo, tt * P:(tt + 1) * P],
                                                 pst[:], AF.Copy)
                    for tt in range(T // P):
                        for nh in range(2):
                            nsl = slice(nh * 512, (nh + 1) * 512)
                            ps = psC.tile([P, 512], f32, tag='out')
                            for ko in range(2):
                                rmm(
                                    ps[:], attT[:, ko, tt * P:(tt + 1) * P],
                                    wo_sb[:, ko, nsl],
                                    start=(ko == 0), stop=(ko == 1))
                            ot = outs.tile([P, 512], f32, tag='ot')
                            nc.vector.tensor_copy(ot[:], ps[:])
                            nc.sync.dma_start(partial[tt, :, nsl], ot[:])

    nc.compile()
    return nc


# ---------------------------------------------------------------- host driver
def _prep_inputs(inputs):
    x = np.ascontiguousarray(np.asarray(inputs['x'], np.float32))
    w1 = np.asarray(inputs['w1'], np.float32)
    b1 = np.asarray(inputs['b1'], np.float32)
    wkr = np.asarray(inputs['wkr'], np.float32)
    bkr = np.asarray(inputs['bkr'], np.float32)
    wqr = np.asarray(inputs['wqr'], np.float32)
    bqr = np.asarray(inputs['bqr'], np.float32)
    wkv = np.asarray(inputs['wkv'], np.float32)
    bkv = np.asarray(inputs['bkv'], np.float32)
    wq = np.asarray(inputs['wq'], np.float32)
    bq = np.asarray(inputs['bq'], np.float32)
    wo = np.asarray(inputs['wo'], np.float32)

    def grp(a, ko):  # [K, M] -> [ko, 128, M]
        return np.ascontiguousarray(a.reshape(ko, P, -1))

    def pack_bias(b):  # [n*128] -> [128, n]
        return np.ascontiguousarray(b.reshape(-1, P).T)

    cos_kr, sin_kr = _rope_tables(DHR)          # [T, 64]
    cos_qr, sin_qr = _rope_tables(DHR * NH)     # [T, 1024]
    ckr_pad = np.zeros((P, T), np.float32)
    skr_pad = np.zeros((P, T), np.float32)
    ckr_pad[64:128] = cos_kr.T
    skr_pad[64:128] = sin_kr.T
    bkr_pad = np.zeros((P, 1), np.float32)
    bkr_pad[64:128, 0] = bkr

    common = {
        'w1': grp(w1, 8), 'b1': pack_bias(b1),
        'wkr': grp(np.concatenate([np.zeros_like(wkr), wkr], axis=1), 8),
        'bkr': bkr_pad,
        'cos_kr': ckr_pad, 'sin_kr': skr_pad,
        'masks': _masks_packed(), 'sperm': _pairswap(),
    }
    in_maps = []
    for core in range(8):
        b, g = divmod(core, 4)
        cols = slice(256 * g, 256 * (g + 1))
        m = dict(common)
        m['xT'] = np.ascontiguousarray(x[b].T.reshape(8, P, T))
        m['wqr'] = grp(wqr[:, cols], 8)
        m['bqr'] = pack_bias(bqr[cols])
        m['wkvk'] = grp(wkv[:, cols], 4)
        m['bkvk'] = pack_bias(bkv[cols])
        m['wkvv'] = grp(wkv[:, 1024 + 256 * g:1024 + 256 * (g + 1)], 4)
        vinit = np.ones((P, 4, 65), np.float32)
        vinit[:, :, 0:64] = bkv[1024 + 256 * g:1024 + 256 * (g + 1)].reshape(1, 4, 64)
        m['vinit'] = vinit
        m['wq'] = grp(wq[:, cols], 4)
        m['bq'] = pack_bias(bq[cols])
        m['wo'] = grp(wo[cols, :], 2)
        m['cos_qr'] = np.ascontiguousarray(cos_qr[:, cols].T.reshape(2, P, T))
        m['sin_qr'] = np.ascontiguousarray(sin_qr[:, cols].T.reshape(2, P, T))
        in_maps.append(m)
    return in_maps


def _run(in_maps, trace=False):
    from concourse.bass_utils import run_bass_kernel_spmd
    key = ('nc', F32R)
    if key not in _BUILT:
        _BUILT[key] = _build_program()
    return run_bass_kernel_spmd(_BUILT[key], in_maps, list(range(8)),
                                trace=trace)


def kernel(**inputs):
    in_maps = _prep_inputs(inputs)
    res = _run(in_maps)
    bo = np.asarray(inputs['bo'], np.float32)
    out = np.zeros((B, T, C), np.float32)
    for core in range(8):
        b = core // 4
        out[b] += res.results[core]['partial'].reshape(T, C)
    out += bo[None, None, :]
    return out.astype(np.asarray(inputs['x']).dtype)

